# revision 7
# baseline (speedup 1.0000x reference)
"""Trainium2 Bass kernel for nn_CosineSimilarityLayer.

out = l2norm_rows(x) @ l2norm_rows_over_N(W)       x:[4096,512]  W:[512,5994]

Math:  out[b,n] = xscale[b] * sum_d x[b,d] * wscale[d] * W[d,n]
  xscale[b] = rsqrt(max(sum_d x[b,d]^2, eps))   (folded into PSUM eviction)
  wscale[d] = rsqrt(max(sum_n W[d,n]^2, eps))   (folded into transposed x)

Sharding: data-parallel over batch (8 cores x [512,512] x-shards, W16
replicated) for the matmul, but COLUMN-sharded for the W-norm scan: each
core squares+reduces only 750 of the (padded) 6000 W columns and a 2KB
AllReduce combines the partial sums.  wscale gates every matmul, so the
8x reduction in norm work pulls the matmul start from ~42us to ~20us.

Host-side layout games (no math on host): W is cast to bf16, padded with
6 zero columns to 6000, and cyclically rotated by 750*core so that each
core's norm shard is its first-arriving 750 columns.  The out tensor is
produced in the same rotated order and un-rotated on the host.

All IO is bf16 (x, W16, out; host casts, out is upcast to fp32 on the
host).  bf16 matmul runs at 1 cycle/row like f32r, but halves every DMA
stream.  Output DMAs are issued from the otherwise-idle GpSimd engine.
"""

import os
import sys
import types
from contextlib import ExitStack

import numpy as np


def _ensure_axon_hooks():
    """bass_utils' trace path imports antenv.axon_hooks, which some images
    lack.  Provide it (wired to the ctypes NTFF hook when available) so
    BASS_TRACE=1 profiles instead of crashing.  No-op when already present."""
    try:
        import antenv.axon_hooks  # noqa: F401
        return
    except ImportError:
        pass
    try:
        import antenv
    except ImportError:
        return
    m = types.ModuleType("antenv.axon_hooks")
    holder = {"h": None}
    m.set_axon_ntff_profile_hook = lambda h: holder.__setitem__("h", h)
    m.get_axon_ntff_profile_hook = lambda: holder["h"]
    sys.modules["antenv.axon_hooks"] = m
    antenv.axon_hooks = m
    try:
        from trn_agent_boot.trn_boot import _ntff_profile_via_ctypes
        so = "/opt/axon/libaxon_pjrt.so"
        if os.path.exists(so):
            m.set_axon_ntff_profile_hook(_ntff_profile_via_ctypes(so))
    except Exception:
        pass


_ensure_axon_hooks()

import ml_dtypes
import concourse.bass as bass
import concourse.tile as tile
from concourse import bacc, mybir
from concourse.bass_utils import run_bass_kernel_spmd
from concourse.masks import make_identity

F32 = mybir.dt.float32
BF16 = mybir.dt.bfloat16
AF = mybir.ActivationFunctionType

B, D, N = 4096, 512, 5994
NP = 6000                  # N padded with 6 zero columns
NCORES = 8
SH = NP // NCORES          # 750-column norm shard per core
P = 128
BSH = B // NCORES          # 512 rows of x per core
BT = BSH // P              # 4 b-tiles
DT = D // P                # 4 d-tiles (contraction)
CHUNK = 500                # output n-chunk (PSUM bank is 512 fp32)
NCH = NP // CHUNK          # 12
GRP = 3                    # chunks per PSUM group (6 mm banks + 2 tp banks)
GW = GRP * CHUNK           # 1500 cols per W16 DMA / out DMA group
NG = NCH // GRP            # 4 groups
EPS = 1e-12


def _build():
    nc = bacc.Bacc("TRN2", target_bir_lowering=False, debug=False,
                   num_devices=NCORES)

    x_d = nc.dram_tensor("x", [BSH, D], BF16, kind="ExternalInput").ap()
    w16_d = nc.dram_tensor("W16", [D, NP], BF16, kind="ExternalInput").ap()
    o_d = nc.dram_tensor("out", [BSH, NP], BF16, kind="ExternalOutput").ap()

    x_r = x_d.rearrange("(t p) d -> p t d", p=P)        # [128, 4, 512]
    w16_r = w16_d.rearrange("(t p) n -> p t n", p=P)    # [128, 4, 6000]
    o_r = o_d.rearrange("(t p) n -> p t n", p=P)        # [128, 4, 6000]

    with tile.TileContext(nc) as tc, ExitStack() as ctx:
        const = ctx.enter_context(tc.tile_pool(name="const", bufs=1))
        xp = ctx.enter_context(tc.tile_pool(name="xp", bufs=1))
        sq = ctx.enter_context(tc.tile_pool(name="sq", bufs=2))
        sc = ctx.enter_context(tc.tile_pool(name="sc", bufs=1))
        xt = ctx.enter_context(tc.tile_pool(name="xt", bufs=1))
        wp = ctx.enter_context(tc.tile_pool(name="wp", bufs=1))
        ostp = ctx.enter_context(tc.tile_pool(name="ostp", bufs=4))
        drp = ctx.enter_context(tc.tile_pool(name="drp", bufs=1, space="DRAM"))
        tp = ctx.enter_context(tc.tile_pool(name="tp", bufs=2, space="PSUM"))
        mm = ctx.enter_context(tc.tile_pool(name="mm", bufs=6, space="PSUM"))

        # ---- input DMAs, issued up front in stream order ----
        # x first (starts the xscale/transpose chain and loads the Sqrt
        # act table early), then W16 by (group, t) slices: 128 lines of
        # 3KB each, so descriptor generation stays cheap.  The core's
        # norm shard is the first 750 columns (host rotated W16).
        x_sb = xp.tile([P, BT, D], BF16)
        nc.sync.dma_start(x_sb, x_r)
        w16 = wp.tile([P, DT, NP], BF16)
        for g in range(NG):
            for t in range(DT):
                nc.sync.dma_start(w16[:, t, g * GW:(g + 1) * GW],
                                  w16_r[:, t, g * GW:(g + 1) * GW])

        # ---- xscale = rsqrt(max(rowsumsq(x), eps)) ----
        xsq = sc.tile([P, BT], F32)
        for bt in range(BT):
            trx = sq.tile([P, D], BF16, tag="trx")
            nc.scalar.activation(trx, x_sb[:, bt, :], AF.Square,
                                 accum_out=xsq[:, bt:bt + 1])
        xmx = sc.tile([P, BT], F32)
        nc.vector.tensor_scalar_max(xmx, xsq, EPS)
        xsr = sc.tile([P, BT], F32)
        nc.scalar.sqrt(xsr, xmx)
        xsc = sc.tile([P, BT], F32)
        nc.vector.reciprocal(xsc, xsr)

        # ---- x^T via PE transpose (bf16, 1 cycle/row) ----
        identity = const.tile([P, P], BF16)
        make_identity(nc, identity)
        xtf = xt.tile([P, DT, BSH], BF16, tag="xtf")
        for dt in range(DT):
            for bt in range(BT):
                pt = tp.tile([P, P], BF16)
                nc.tensor.transpose(pt, x_sb[:, bt, dt * P:(dt + 1) * P],
                                    identity)
                nc.vector.tensor_copy(xtf[:, dt, bt * P:(bt + 1) * P], pt)

        # ---- W norm partials over the local 750-column shard ----
        # ACT: fused Square+accum for t0/t1.  DVE: square then reduce
        # (both passes at the 2x 16-bit rate) for t2/t3.
        S = sc.tile([P, DT], F32)
        for t in range(2):
            tra = sq.tile([P, SH], BF16, tag="tra")
            nc.scalar.activation(tra, w16[:, t, :SH], AF.Square,
                                 accum_out=S[:, t:t + 1])
        trd = sq.tile([P, 2, SH], BF16, tag="trd")
        nc.vector.tensor_tensor(trd, w16[:, 2:4, :SH], w16[:, 2:4, :SH],
                                mybir.AluOpType.mult)
        nc.vector.reduce_sum(S[:, 2:4], trd, axis=mybir.AxisListType.X)

        # ---- AllReduce the [128,4] partials across the 8 cores ----
        s_in = drp.tile([P, DT], F32, name="s_in")
        s_out = drp.tile([P, DT], F32, name="s_out")
        nc.gpsimd.dma_start(s_in, S)
        nc.gpsimd.collective_compute(
            "AllReduce", mybir.AluOpType.add,
            replica_groups=[list(range(NCORES))],
            ins=[s_in.opt()], outs=[s_out.opt()])
        Sg = sc.tile([P, DT], F32)
        nc.gpsimd.dma_start(Sg, s_out)

        # ---- wscale = rsqrt(max(S, eps)) ----
        wmx = sc.tile([P, DT], F32)
        nc.vector.tensor_scalar_max(wmx, Sg, EPS)
        wsr = sc.tile([P, DT], F32)
        nc.scalar.sqrt(wsr, wmx)
        wsc = sc.tile([P, DT], F32)
        nc.vector.reciprocal(wsc, wsr)

        # ---- fold wscale into x^T ----
        xtr = xt.tile([P, DT, BSH], BF16, tag="xtr")
        for dt in range(DT):
            nc.vector.tensor_scalar_mul(xtr[:, dt, :], xtf[:, dt, :],
                                        wsc[:, dt:dt + 1])

        # ---- matmul: group outer (W arrival order), dt outer within a
        # group so the stationary operand is reused across banks ----
        evict = [0]
        for bt in range(BT):
            for g in range(NG):
                pss = [mm.tile([P, CHUNK], F32, tag="ps", name=f"ps{c}")
                       for c in range(GRP)]
                for dt in range(DT):
                    for c in range(GRP):
                        n0 = g * GW + c * CHUNK
                        nc.tensor.matmul(
                            pss[c],
                            xtr[:, dt, bt * P:(bt + 1) * P],
                            w16[:, dt, n0:n0 + CHUNK],
                            start=(dt == 0), stop=(dt == DT - 1))
                ost = ostp.tile([P, GW], BF16, tag="ost")
                for c in range(GRP):
                    # GPSIMD cannot read PSUM: alternate ACT/DVE.
                    dst = ost[:, c * CHUNK:(c + 1) * CHUNK]
                    if evict[0] % 2 == 0:
                        nc.scalar.activation(dst, pss[c], AF.Copy,
                                             scale=xsc[:, bt:bt + 1])
                    else:
                        nc.vector.tensor_scalar_mul(dst, pss[c],
                                                    xsc[:, bt:bt + 1])
                    evict[0] += 1
                nc.gpsimd.dma_start(o_r[:, bt, g * GW:(g + 1) * GW], ost)

    nc.compile()
    return nc


LAST_RESULT = None


def kernel(x: np.ndarray, W: np.ndarray) -> np.ndarray:
    global LAST_RESULT
    x = np.ascontiguousarray(x, dtype=np.float32)
    W = np.ascontiguousarray(W, dtype=np.float32)
    assert x.shape == (B, D) and W.shape == (D, N)

    x16 = x.astype(ml_dtypes.bfloat16)
    W16 = np.zeros((D, NP), dtype=ml_dtypes.bfloat16)
    W16[:, :N] = W.astype(ml_dtypes.bfloat16)

    nc = _build()

    in_maps = []
    for c in range(NCORES):
        w_rot = np.ascontiguousarray(np.roll(W16, -SH * c, axis=1))
        in_maps.append({"x": np.ascontiguousarray(x16[c * BSH:(c + 1) * BSH]),
                        "W16": w_rot})

    res = run_bass_kernel_spmd(nc, in_maps, core_ids=list(range(NCORES)))
    LAST_RESULT = res
    outs = []
    for c in range(NCORES):
        o = np.roll(res.results[c]["out"], SH * c, axis=1)
        outs.append(o[:, :N])
    return np.concatenate(outs, axis=0).astype(np.float32)


# revision 8
# speedup vs baseline: 1.4709x; 1.4709x over previous
"""Trainium2 Bass kernel for nn_CosineSimilarityLayer.

out = l2norm_rows(x) @ l2norm_rows_over_N(W)       x:[4096,512]  W:[512,5994]

Math:  out[b,n] = xscale[b] * sum_d x[b,d] * wscale[d] * W[d,n]
  xscale[b] = rsqrt(max(sum_d x[b,d]^2, eps))   (folded into PSUM eviction)
  wscale[d] = rsqrt(max(sum_n W[d,n]^2, eps))   (folded into transposed x)

Sharding: data-parallel over batch - 8 cores x [512, 512] x-shards, W
replicated.  No collectives: a measured 2KB AllReduce costs ~40us on
this system (NEFF-start skew + CC latency), far more than the redundant
per-core norm work it would save.

Pipeline (all IO bf16; host casts, out upcast on host):
  * wscale gates every matmul, so W is streamed twice: a 3MB fp8e4m3
    "norm shadow" first, squared as it lands by ACT (fused Square+accum)
    + DVE (square, reduce) + Pool (square; DVE reduces) with spans sized
    from measured engine rates; then the 6.1MB bf16 W for the matmul.
    fp8 norm error is ~1e-3 relative on wscale -> ~1e-4 on out (gate 2e-2).
  * x lands first: DVE computes xscale (2-pass) and drains the PE
    transposes while ACT/Pool wait for the shadow.
  * both ACT tables (Square, Sqrt) are preloaded with dummy ops before
    any data lands, keeping table loads off the critical path.
  * matmul: group-outer / bt-inner so W chunks are consumed ~3x slower
    than they land; PSUM groups of 3 banks, 6 bufs (double-buffered);
    dt-outer inside a group reuses the stationary across banks.
  * PSUM eviction (scale by xscale, round to bf16) alternates ACT/DVE;
    out DMA per (group, bt) with 3KB lines, issued from idle GpSimd.
"""

import os
import sys
import types
from contextlib import ExitStack

import numpy as np


def _ensure_axon_hooks():
    """bass_utils' trace path imports antenv.axon_hooks, which some images
    lack.  Provide it (wired to the ctypes NTFF hook when available) so
    BASS_TRACE=1 profiles instead of crashing.  No-op when already present."""
    try:
        import antenv.axon_hooks  # noqa: F401
        return
    except ImportError:
        pass
    try:
        import antenv
    except ImportError:
        return
    m = types.ModuleType("antenv.axon_hooks")
    holder = {"h": None}
    m.set_axon_ntff_profile_hook = lambda h: holder.__setitem__("h", h)
    m.get_axon_ntff_profile_hook = lambda: holder["h"]
    sys.modules["antenv.axon_hooks"] = m
    antenv.axon_hooks = m
    try:
        from trn_agent_boot.trn_boot import _ntff_profile_via_ctypes
        so = "/opt/axon/libaxon_pjrt.so"
        if os.path.exists(so):
            m.set_axon_ntff_profile_hook(_ntff_profile_via_ctypes(so))
    except Exception:
        pass


_ensure_axon_hooks()

import ml_dtypes
import concourse.bass as bass
import concourse.tile as tile
from concourse import bacc, mybir
from concourse.bass_utils import run_bass_kernel_spmd
from concourse.masks import make_identity

F32 = mybir.dt.float32
BF16 = mybir.dt.bfloat16
FP8 = mybir.dt.float8e4
AF = mybir.ActivationFunctionType

B, D, N = 4096, 512, 5994
NCORES = 8
P = 128
BSH = B // NCORES          # 512 rows of x per core
BT = BSH // P              # 4 b-tiles
DT = D // P                # 4 d-tiles (contraction)
CHUNK = 512                # output n-chunk (one PSUM bank of fp32)
GRP = 3                    # chunks per PSUM group (6 mm banks + 2 tp banks)
EPS = 1e-12

CHUNKS = []
_n0 = 0
while _n0 < N:
    CHUNKS.append((_n0, min(CHUNK, N - _n0)))
    _n0 += CHUNK
NCH = len(CHUNKS)          # 12
GROUPS = []                # (start, width) of GRP-chunk groups
for _g in range(0, NCH, GRP):
    _c = CHUNKS[_g:_g + GRP]
    GROUPS.append((_c[0][0], _c[-1][0] + _c[-1][1] - _c[0][0]))

# norm-shadow squaring spans per engine, from measured rates:
# ACT 0.833 ns/col (fused, +315ns/instr), DVE 1.56 (fp8 square + bf16
# reduce), Pool 1.84 (square only, measured; DVE reduces its output).
_SPLIT = (3200, 1150, 1644)
assert sum(_SPLIT) == N
_SPANS = []
_c0 = 0
for _w in _SPLIT:
    _SPANS.append((_c0, _w))
    _c0 += _w


def _build():
    nc = bacc.Bacc("TRN2", target_bir_lowering=False, debug=False,
                   num_devices=NCORES)

    x_d = nc.dram_tensor("x", [BSH, D], BF16, kind="ExternalInput").ap()
    w16_d = nc.dram_tensor("W16", [D, N], BF16, kind="ExternalInput").ap()
    w8_d = nc.dram_tensor("W8", [D, N], FP8, kind="ExternalInput").ap()
    o_d = nc.dram_tensor("out", [BSH, N], BF16, kind="ExternalOutput").ap()

    x_r = x_d.rearrange("(t p) d -> p t d", p=P)        # [128, 4, 512]
    w16_r = w16_d.rearrange("(t p) n -> p t n", p=P)    # [128, 4, 5994]
    w8_r = w8_d.rearrange("(t p) n -> p t n", p=P)      # [128, 4, 5994]
    o_r = o_d.rearrange("(t p) n -> p t n", p=P)        # [128, 4, 5994]

    with tile.TileContext(nc) as tc, ExitStack() as ctx:
        const = ctx.enter_context(tc.tile_pool(name="const", bufs=1))
        xp = ctx.enter_context(tc.tile_pool(name="xp", bufs=1))
        sq = ctx.enter_context(tc.tile_pool(name="sq", bufs=2))
        sc = ctx.enter_context(tc.tile_pool(name="sc", bufs=1))
        xt = ctx.enter_context(tc.tile_pool(name="xt", bufs=1))
        wp = ctx.enter_context(tc.tile_pool(name="wp", bufs=1))
        ostp = ctx.enter_context(tc.tile_pool(name="ostp", bufs=4))
        tp = ctx.enter_context(tc.tile_pool(name="tp", bufs=2, space="PSUM"))
        mm = ctx.enter_context(tc.tile_pool(name="mm", bufs=6, space="PSUM"))

        # ---- input DMAs, issued up front in stream order ----
        x_sb = xp.tile([P, BT, D], BF16)
        nc.sync.dma_start(x_sb, x_r)
        w8 = wp.tile([P, DT, N], FP8)
        for t in range(DT):
            nc.sync.dma_start(w8[:, t, :], w8_r[:, t, :])
        w16 = wp.tile([P, DT, N], BF16)
        for g0, gw in GROUPS:
            for t in range(DT):
                nc.sync.dma_start(w16[:, t, g0:g0 + gw],
                                  w16_r[:, t, g0:g0 + gw])

        # ---- preload both ACT tables before any data lands ----
        dum = sc.tile([P, 2], F32)
        dum2 = sc.tile([P, 2], F32)
        nc.scalar.activation(dum[:, 0:1], dum[:, 1:2], AF.Square)
        nc.scalar.activation(dum2[:, 0:1], dum[:, 0:1], AF.Sqrt)

        # ---- xscale = rsqrt(max(rowsumsq(x), eps)) on DVE (2-pass) ----
        xsqt = sq.tile([P, BT, D], BF16, tag="xsqt")
        nc.vector.tensor_tensor(xsqt, x_sb, x_sb, mybir.AluOpType.mult)
        xsq = sc.tile([P, BT], F32)
        nc.vector.reduce_sum(xsq, xsqt, axis=mybir.AxisListType.X)
        xmx = sc.tile([P, BT], F32)
        nc.vector.tensor_scalar_max(xmx, xsq, EPS)
        xsr = sc.tile([P, BT], F32)
        nc.scalar.sqrt(xsr, xmx)
        xsc = sc.tile([P, BT], F32)
        nc.vector.reciprocal(xsc, xsr)

        # ---- x^T via PE transpose (bf16, 1 cycle/row) ----
        identity = const.tile([P, P], BF16)
        make_identity(nc, identity)
        xtf = xt.tile([P, DT, BSH], BF16, tag="xtf")
        for dt in range(DT):
            for bt in range(BT):
                pt = tp.tile([P, P], BF16)
                nc.tensor.transpose(pt, x_sb[:, bt, dt * P:(dt + 1) * P],
                                    identity)
                nc.vector.tensor_copy(xtf[:, dt, bt * P:(bt + 1) * P], pt)

        # ---- W norm partials from the fp8 shadow ----
        wsqp = sc.tile([P, DT, 3], F32)
        for t in range(DT):
            a0, aw = _SPANS[0]
            tra = sq.tile([P, _SPLIT[0]], BF16, tag="tra")
            nc.scalar.activation(tra, w8[:, t, a0:a0 + aw], AF.Square,
                                 accum_out=wsqp[:, t, 0:1])
            d0, dw = _SPANS[1]
            trd = sq.tile([P, _SPLIT[1]], BF16, tag="trd")
            nc.vector.tensor_tensor(trd, w8[:, t, d0:d0 + dw],
                                    w8[:, t, d0:d0 + dw],
                                    mybir.AluOpType.mult)
            nc.vector.reduce_sum(wsqp[:, t, 1:2], trd,
                                 axis=mybir.AxisListType.X)
            p0, pw = _SPANS[2]
            trp = sq.tile([P, _SPLIT[2]], BF16, tag="trp")
            nc.gpsimd.tensor_tensor(trp, w8[:, t, p0:p0 + pw],
                                    w8[:, t, p0:p0 + pw],
                                    mybir.AluOpType.mult)
            nc.vector.reduce_sum(wsqp[:, t, 2:3], trp,
                                 axis=mybir.AxisListType.X)

        # ---- wscale = rsqrt(max(sum(partials), eps)) ----
        wsq = sc.tile([P, DT, 1], F32)
        nc.vector.reduce_sum(wsq, wsqp, axis=mybir.AxisListType.X)
        wmx = sc.tile([P, DT, 1], F32)
        nc.vector.tensor_scalar_max(wmx, wsq, EPS)
        wsr = sc.tile([P, DT, 1], F32)
        nc.scalar.sqrt(wsr, wmx)
        wsc = sc.tile([P, DT, 1], F32)
        nc.vector.reciprocal(wsc, wsr)

        # ---- fold wscale into x^T ----
        xtr = xt.tile([P, DT, BSH], BF16, tag="xtr")
        for dt in range(DT):
            nc.vector.tensor_scalar_mul(xtr[:, dt, :], xtf[:, dt, :],
                                        wsc[:, dt, :])

        # ---- matmul: group outer (W arrival order), bt inner; dt outer
        # within a group so the stationary is reused across banks ----
        evict = [0]
        for g, (g0, gw) in enumerate(GROUPS):
            grp = CHUNKS[g * GRP:(g + 1) * GRP]
            for bt in range(BT):
                pss = [mm.tile([P, CHUNK], F32, tag="ps", name=f"ps{c}")
                       for c in range(len(grp))]
                for dt in range(DT):
                    for c, (n0, nw) in enumerate(grp):
                        nc.tensor.matmul(
                            pss[c][:, :nw],
                            xtr[:, dt, bt * P:(bt + 1) * P],
                            w16[:, dt, n0:n0 + nw],
                            start=(dt == 0), stop=(dt == DT - 1))
                ost = ostp.tile([P, GRP * CHUNK], BF16, tag="ost")
                for c, (n0, nw) in enumerate(grp):
                    # GPSIMD cannot read PSUM: alternate ACT/DVE.
                    dst = ost[:, n0 - g0:n0 - g0 + nw]
                    if evict[0] % 2 == 0:
                        nc.scalar.activation(dst, pss[c][:, :nw], AF.Copy,
                                             scale=xsc[:, bt:bt + 1])
                    else:
                        nc.vector.tensor_scalar_mul(dst, pss[c][:, :nw],
                                                    xsc[:, bt:bt + 1])
                    evict[0] += 1
                nc.gpsimd.dma_start(o_r[:, bt, g0:g0 + gw], ost[:, :gw])

    nc.compile()
    return nc


LAST_RESULT = None


def kernel(x: np.ndarray, W: np.ndarray) -> np.ndarray:
    global LAST_RESULT
    x = np.ascontiguousarray(x, dtype=np.float32)
    W = np.ascontiguousarray(W, dtype=np.float32)
    assert x.shape == (B, D) and W.shape == (D, N)

    x16 = x.astype(ml_dtypes.bfloat16)
    W16 = np.ascontiguousarray(W.astype(ml_dtypes.bfloat16))
    W8 = np.ascontiguousarray(W.astype(ml_dtypes.float8_e4m3))

    nc = _build()

    in_maps = [{"x": np.ascontiguousarray(x16[c * BSH:(c + 1) * BSH]),
                "W16": W16, "W8": W8}
               for c in range(NCORES)]

    res = run_bass_kernel_spmd(nc, in_maps, core_ids=list(range(NCORES)))
    LAST_RESULT = res
    out = np.concatenate([res.results[c]["out"] for c in range(NCORES)],
                         axis=0)
    return out.astype(np.float32)


# revision 17
# speedup vs baseline: 1.6257x; 1.1053x over previous
"""Trainium2 Bass kernel for nn_CosineSimilarityLayer.

out = l2norm_rows(x) @ l2norm_rows_over_N(W)       x:[4096,512]  W:[512,5994]

Math:  out[b,n] = xscale[b] * sum_d x[b,d] * wscale[d] * W[d,n]
  xscale[b] = rsqrt(max(sum_d x[b,d]^2, eps))   (folded into PSUM eviction)
  wscale[d] = rsqrt(max(sum_n W[d,n]^2, eps))   (folded into transposed x)

Sharding: data-parallel over batch - 8 cores x [512, 512] x-shards, W
replicated.  No collectives: a measured 2KB AllReduce costs ~40us here.

wscale gates every matmul, and the W-norm scan is too slow on the
vector engines (~14us of ACT/DVE/Pool time).  Instead the PE computes
it as a Gram diagonal: the host sends a transposed fp8e4m3 shadow of W
(pre-interleaved for DoubleRow), and the PE accumulates the 4 diagonal
128x128 blocks of W8T^T @ W8T over all 24 row-tile pairs - psum
diagonals are then sum_n W[d,n]^2.  A DVE identity-mask + reduce
extracts the diagonals.  Self-products make DoubleRow interleaving
correctness-free, and the PE (idle while waiting for wscale anyway)
chases the 3MB shadow DMA at line rate, so wscale is ready ~2us after
the shadow lands.  fp8 norm error is ~1e-3 relative on wscale ->
~1e-4 on out (gate 2e-2).

All IO bf16 (host casts, out upcast on host).  Matmul: group-outer /
bt-inner so W chunks are consumed ~3x slower than they land; PSUM
groups of 3 banks, 6 bufs; dt-outer inside a group reuses the
stationary.  Eviction alternates ACT/DVE; out DMA issued from GpSimd.
"""

import os
import sys
import types
from contextlib import ExitStack

import numpy as np


def _ensure_axon_hooks():
    """bass_utils' trace path imports antenv.axon_hooks, which some images
    lack.  Provide it (wired to the ctypes NTFF hook when available) so
    BASS_TRACE=1 profiles instead of crashing.  No-op when already present."""
    try:
        import antenv.axon_hooks  # noqa: F401
        return
    except ImportError:
        pass
    try:
        import antenv
    except ImportError:
        return
    m = types.ModuleType("antenv.axon_hooks")
    holder = {"h": None}
    m.set_axon_ntff_profile_hook = lambda h: holder.__setitem__("h", h)
    m.get_axon_ntff_profile_hook = lambda: holder["h"]
    sys.modules["antenv.axon_hooks"] = m
    antenv.axon_hooks = m
    try:
        from trn_agent_boot.trn_boot import _ntff_profile_via_ctypes
        so = "/opt/axon/libaxon_pjrt.so"
        if os.path.exists(so):
            m.set_axon_ntff_profile_hook(_ntff_profile_via_ctypes(so))
    except Exception:
        pass


_ensure_axon_hooks()

import ml_dtypes
import concourse.bass as bass
import concourse.tile as tile
from concourse import bacc, mybir
from concourse.bass_utils import run_bass_kernel_spmd
from concourse.masks import make_identity

F32 = mybir.dt.float32
BF16 = mybir.dt.bfloat16
FP8 = mybir.dt.float8e4
AF = mybir.ActivationFunctionType

B, D, N = 4096, 512, 5994
NCORES = 8
P = 128
BSH = B // NCORES          # 512 rows of x per core
BT = BSH // P              # 4 b-tiles
DT = D // P                # 4 d-tiles (contraction)
CHUNK = 512                # output n-chunk (one PSUM bank of fp32)
GRP = 3                    # chunks per PSUM group (6 mm banks + 2 tp banks)
EPS = 1e-12

NPAIR = 24                 # W8T row-tile pairs: 24*256 = 6144 >= 5994
NTP = NPAIR * 2 * P        # padded row count (6144)

CHUNKS = []
_n0 = 0
while _n0 < N:
    CHUNKS.append((_n0, min(CHUNK, N - _n0)))
    _n0 += CHUNK
NCH = len(CHUNKS)          # 12
GROUPS = []                # (start, width) of GRP-chunk groups
for _g in range(0, NCH, GRP):
    _c = CHUNKS[_g:_g + GRP]
    GROUPS.append((_c[0][0], _c[-1][0] + _c[-1][1] - _c[0][0]))


def _build():
    nc = bacc.Bacc("TRN2", target_bir_lowering=False, debug=False,
                   num_devices=NCORES)

    x_d = nc.dram_tensor("x", [BSH, D], BF16, kind="ExternalInput").ap()
    w16_d = nc.dram_tensor("W16", [D, N], BF16, kind="ExternalInput").ap()
    # transposed fp8 shadow, host-interleaved to [128, pair, 2, D]
    w8t_d = nc.dram_tensor("W8T", [P, NPAIR, 2, D], FP8,
                           kind="ExternalInput").ap()
    o_d = nc.dram_tensor("out", [BSH, N], BF16, kind="ExternalOutput").ap()
    sdbg_d = nc.dram_tensor("SDBG", [P, DT], F32, kind="ExternalOutput").ap()

    x_r = x_d.rearrange("(t p) d -> p t d", p=P)        # [128, 4, 512]
    w16_r = w16_d.rearrange("(t p) n -> p t n", p=P)    # [128, 4, 5994]
    o_r = o_d.rearrange("(t p) n -> p t n", p=P)        # [128, 4, 5994]

    with tile.TileContext(nc) as tc, ExitStack() as ctx:
        const = ctx.enter_context(tc.tile_pool(name="const", bufs=1))
        xp = ctx.enter_context(tc.tile_pool(name="xp", bufs=1))
        sq = ctx.enter_context(tc.tile_pool(name="sq", bufs=2))
        sc = ctx.enter_context(tc.tile_pool(name="sc", bufs=1))
        xt = ctx.enter_context(tc.tile_pool(name="xt", bufs=1))
        wp = ctx.enter_context(tc.tile_pool(name="wp", bufs=1))
        ostp = ctx.enter_context(tc.tile_pool(name="ostp", bufs=4))
        tp = ctx.enter_context(tc.tile_pool(name="tp", bufs=1, space="PSUM"))
        gp = ctx.enter_context(tc.tile_pool(name="gp", bufs=1, space="PSUM"))
        mm = ctx.enter_context(tc.tile_pool(name="mm", bufs=6, space="PSUM"))

        # ---- input DMAs, issued up front in stream order ----
        # W8T first: the Gram (and so wscale) is the longest dependency
        # chain; x next for transposes; W16 groups last.
        w8t = wp.tile([P, NPAIR, 2, D], FP8)
        for s in range(4):
            j0 = s * (NPAIR // 4)
            nc.sync.dma_start(w8t[:, j0:j0 + NPAIR // 4],
                              w8t_d[:, j0:j0 + NPAIR // 4])
        x_sb = xp.tile([P, BT, D], BF16)
        nc.sync.dma_start(x_sb, x_r)
        w16 = wp.tile([P, DT, N], BF16)
        for g0, gw in GROUPS:
            for t in range(DT):
                nc.sync.dma_start(w16[:, t, g0:g0 + gw],
                                  w16_r[:, t, g0:g0 + gw])

        # ---- preload both ACT tables before any data lands ----
        dum = sc.tile([P, 2], F32)
        dum2 = sc.tile([P, 2], F32)
        nc.scalar.activation(dum[:, 0:1], dum[:, 1:2], AF.Square)
        nc.scalar.activation(dum2[:, 0:1], dum[:, 0:1], AF.Sqrt)

        # ---- xscale = rsqrt(max(rowsumsq(x), eps)) on DVE (2-pass) ----
        xsqt = sq.tile([P, BT, D], BF16, tag="xsqt")
        nc.vector.tensor_tensor(xsqt, x_sb, x_sb, mybir.AluOpType.mult)
        xsq = sc.tile([P, BT], F32)
        nc.vector.reduce_sum(xsq, xsqt, axis=mybir.AxisListType.X)
        xmx = sc.tile([P, BT], F32)
        nc.vector.tensor_scalar_max(xmx, xsq, EPS)
        xsr = sc.tile([P, BT], F32)
        nc.scalar.sqrt(xsr, xmx)
        xsc = sc.tile([P, BT], F32)
        nc.vector.reciprocal(xsc, xsr)

        # ---- x^T via PE transpose (bf16, 1 cycle/row) ----
        identity = const.tile([P, P], BF16)
        make_identity(nc, identity)
        xtf = xt.tile([P, DT, BSH], BF16, tag="xtf")
        for dt in range(DT):
            for bt in range(BT):
                pt = tp.tile([P, P], BF16)
                nc.tensor.transpose(pt, x_sb[:, bt, dt * P:(dt + 1) * P],
                                    identity)
                nc.vector.tensor_copy(xtf[:, dt, bt * P:(bt + 1) * P], pt)

        # ---- W norms: PE Gram diagonal over the fp8 shadow ----
        # gps[db][i,j] accumulates sum_n W8T[n, db*128+i] * W8T[n, db*128+j];
        # its diagonal is sum_n W[d,n]^2 for d = db*128 + i.
        # PSUM start=True zeroing is bank-granular (2KB), so four 512B
        # accumulation regions in one bank cannot each use start=True:
        # every later start wipes the earlier regions' first pair.
        # Pre-zero the bank once and accumulate with start=False.
        gps = gp.tile([P, DT, P], F32)
        nc.vector.memset(gps, 0.0)
        for j in range(NPAIR):
            for db in range(DT):
                blk = w8t[:, j, :, db * P:(db + 1) * P]
                nc.tensor.matmul(gps[:, db, :], blk, blk,
                                 perf_mode=mybir.MatmulPerfMode.DoubleRow,
                                 start=False, stop=(j == NPAIR - 1),
                                 skip_group_check=True)
        wsq = sc.tile([P, DT], F32)
        for db in range(DT):
            dg = sq.tile([P, P], F32, tag="diag", name=f"dg{db}")
            nc.vector.tensor_tensor(dg, gps[:, db, :], identity,
                                    mybir.AluOpType.mult)
            nc.vector.reduce_sum(wsq[:, db:db + 1], dg,
                                 axis=mybir.AxisListType.X)
        nc.gpsimd.dma_start(sdbg_d, wsq)

        # ---- wscale = rsqrt(max(S, eps)) ----
        wmx = sc.tile([P, DT], F32)
        nc.vector.tensor_scalar_max(wmx, wsq, EPS)
        wsr = sc.tile([P, DT], F32)
        nc.scalar.sqrt(wsr, wmx)
        wsc = sc.tile([P, DT], F32)
        nc.vector.reciprocal(wsc, wsr)

        # ---- fold wscale into x^T ----
        xtr = xt.tile([P, DT, BSH], BF16, tag="xtr")
        for dt in range(DT):
            nc.vector.tensor_scalar_mul(xtr[:, dt, :], xtf[:, dt, :],
                                        wsc[:, dt:dt + 1])

        # ---- matmul: group outer (W arrival order), bt inner; dt outer
        # within a group so the stationary is reused across banks ----
        evict = [0]
        for g, (g0, gw) in enumerate(GROUPS):
            grp = CHUNKS[g * GRP:(g + 1) * GRP]
            for bt in range(BT):
                pss = [mm.tile([P, CHUNK], F32, tag="ps", name=f"ps{c}")
                       for c in range(len(grp))]
                for dt in range(DT):
                    for c, (n0, nw) in enumerate(grp):
                        nc.tensor.matmul(
                            pss[c][:, :nw],
                            xtr[:, dt, bt * P:(bt + 1) * P],
                            w16[:, dt, n0:n0 + nw],
                            start=(dt == 0), stop=(dt == DT - 1))
                ost = ostp.tile([P, GRP * CHUNK], BF16, tag="ost")
                for c, (n0, nw) in enumerate(grp):
                    # GPSIMD cannot read PSUM: alternate ACT/DVE.
                    dst = ost[:, n0 - g0:n0 - g0 + nw]
                    if evict[0] % 2 == 0:
                        nc.scalar.activation(dst, pss[c][:, :nw], AF.Copy,
                                             scale=xsc[:, bt:bt + 1])
                    else:
                        nc.vector.tensor_scalar_mul(dst, pss[c][:, :nw],
                                                    xsc[:, bt:bt + 1])
                    evict[0] += 1
                nc.gpsimd.dma_start(o_r[:, bt, g0:g0 + gw], ost[:, :gw])

    nc.compile()
    return nc


LAST_RESULT = None


def kernel(x: np.ndarray, W: np.ndarray) -> np.ndarray:
    global LAST_RESULT
    x = np.ascontiguousarray(x, dtype=np.float32)
    W = np.ascontiguousarray(W, dtype=np.float32)
    assert x.shape == (B, D) and W.shape == (D, N)

    x16 = x.astype(ml_dtypes.bfloat16)
    W16 = np.ascontiguousarray(W.astype(ml_dtypes.bfloat16))

    # transposed fp8 shadow [N, D] -> pad to 6144 rows -> interleave to
    # [128, pair, 2, D]: partition p of pair j holds rows 256j+p and
    # 256j+128+p (DoubleRow k-tile pairing; self-products make the
    # pairing convention irrelevant).
    w8t = np.zeros((NTP, D), dtype=ml_dtypes.float8_e4m3)
    w8t[:N] = W.T.astype(ml_dtypes.float8_e4m3)
    w8t = np.ascontiguousarray(
        w8t.reshape(NPAIR, 2, P, D).transpose(2, 0, 1, 3))

    nc = _build()

    in_maps = [{"x": np.ascontiguousarray(x16[c * BSH:(c + 1) * BSH]),
                "W16": W16, "W8T": w8t}
               for c in range(NCORES)]

    res = run_bass_kernel_spmd(nc, in_maps, core_ids=list(range(NCORES)))
    LAST_RESULT = res
    out = np.concatenate([res.results[c]["out"] for c in range(NCORES)],
                         axis=0)
    return out.astype(np.float32)


# revision 22
# speedup vs baseline: 1.6425x; 1.0103x over previous
"""Trainium2 Bass kernel for nn_CosineSimilarityLayer.

out = l2norm_rows(x) @ l2norm_rows_over_N(W)       x:[4096,512]  W:[512,5994]

Math:  out[b,n] = xscale[b] * sum_d x[b,d] * wscale[d] * W[d,n]
  xscale[b] = rsqrt(max(sum_d x[b,d]^2, eps))   (folded into PSUM eviction)
  wscale[d] = rsqrt(max(sum_n W[d,n]^2, eps))   (folded into transposed x)

Sharding: data-parallel over batch - 8 cores x [512, 512] x-shards, W
replicated.  No collectives: a measured 2KB AllReduce costs ~40us here.

wscale gates every matmul, and the W-norm scan is too slow on the
vector engines (~14us of ACT/DVE/Pool time).  Instead the PE computes
it as a Gram diagonal: the host sends a transposed fp8e4m3 shadow of W
(pre-interleaved for DoubleRow), and the PE accumulates the 4 diagonal
128x128 blocks of W8T^T @ W8T over all 24 row-tile pairs - psum
diagonals are then sum_n W[d,n]^2.  A DVE identity-mask + reduce
extracts the diagonals.  Self-products make DoubleRow interleaving
correctness-free, and the PE (idle while waiting for wscale anyway)
chases the 3MB shadow DMA at line rate, so wscale is ready ~2us after
the shadow lands.  fp8 norm error is ~1e-3 relative on wscale ->
~1e-4 on out (gate 2e-2).

All IO bf16 (host casts, out upcast on host).  Matmul: group-outer /
bt-inner so W chunks are consumed ~3x slower than they land; PSUM
groups of 3 banks, 6 bufs; dt-outer inside a group reuses the
stationary.  Eviction alternates ACT/DVE; out DMA issued from GpSimd.
"""

import os
import sys
import types
from contextlib import ExitStack

import numpy as np


def _ensure_axon_hooks():
    """bass_utils' trace path imports antenv.axon_hooks, which some images
    lack.  Provide it (wired to the ctypes NTFF hook when available) so
    BASS_TRACE=1 profiles instead of crashing.  No-op when already present."""
    try:
        import antenv.axon_hooks  # noqa: F401
        return
    except ImportError:
        pass
    try:
        import antenv
    except ImportError:
        return
    m = types.ModuleType("antenv.axon_hooks")
    holder = {"h": None}
    m.set_axon_ntff_profile_hook = lambda h: holder.__setitem__("h", h)
    m.get_axon_ntff_profile_hook = lambda: holder["h"]
    sys.modules["antenv.axon_hooks"] = m
    antenv.axon_hooks = m
    try:
        from trn_agent_boot.trn_boot import _ntff_profile_via_ctypes
        so = "/opt/axon/libaxon_pjrt.so"
        if os.path.exists(so):
            m.set_axon_ntff_profile_hook(_ntff_profile_via_ctypes(so))
    except Exception:
        pass


_ensure_axon_hooks()

import ml_dtypes
import concourse.bass as bass
import concourse.tile as tile
from concourse import bacc, mybir
from concourse.bass_utils import run_bass_kernel_spmd
from concourse.masks import make_identity

F32 = mybir.dt.float32
BF16 = mybir.dt.bfloat16
FP8 = mybir.dt.float8e4
AF = mybir.ActivationFunctionType

B, D, N = 4096, 512, 5994
NCORES = 8
P = 128
BSH = B // NCORES          # 512 rows of x per core
BT = BSH // P              # 4 b-tiles
DT = D // P                # 4 d-tiles (contraction)
CHUNK = 512                # output n-chunk (one PSUM bank of fp32)
GRP = 3                    # chunks per PSUM group (6 mm banks + 2 tp banks)
EPS = 1e-12

NPAIR = 24                 # W8T row-tile pairs: 24*256 = 6144 >= 5994
NTP = NPAIR * 2 * P        # padded row count (6144)

CHUNKS = []
_n0 = 0
while _n0 < N:
    CHUNKS.append((_n0, min(CHUNK, N - _n0)))
    _n0 += CHUNK
NCH = len(CHUNKS)          # 12
# chunk-group partition: tiny leading groups so the first matmul only
# waits on 1 chunk of W16, then steady GRP-chunk groups
_GIDX = [[0], [1, 2]]
_g = 3
while _g < NCH:
    _GIDX.append(list(range(_g, min(_g + GRP, NCH))))
    _g += GRP
GROUPS = []                # (start, width, [chunk indices])
for _ix in _GIDX:
    _c = [CHUNKS[i] for i in _ix]
    GROUPS.append((_c[0][0], _c[-1][0] + _c[-1][1] - _c[0][0], _ix))


def _build():
    nc = bacc.Bacc("TRN2", target_bir_lowering=False, debug=False,
                   num_devices=NCORES)

    x_d = nc.dram_tensor("x", [BSH, D], BF16, kind="ExternalInput").ap()
    w16_d = nc.dram_tensor("W16", [D, N], BF16, kind="ExternalInput").ap()
    # transposed fp8 shadow, host-interleaved to [128, pair, 2, D]
    w8t_d = nc.dram_tensor("W8T", [P, NPAIR, 2, D], FP8,
                           kind="ExternalInput").ap()
    o_d = nc.dram_tensor("out", [BSH, N], BF16, kind="ExternalOutput").ap()
    sdbg_d = nc.dram_tensor("SDBG", [P, DT], F32, kind="ExternalOutput").ap()

    x_r = x_d.rearrange("(t p) d -> p t d", p=P)        # [128, 4, 512]
    w16_r = w16_d.rearrange("(t p) n -> p t n", p=P)    # [128, 4, 5994]
    o_r = o_d.rearrange("(t p) n -> p t n", p=P)        # [128, 4, 5994]

    with tile.TileContext(nc) as tc, ExitStack() as ctx:
        const = ctx.enter_context(tc.tile_pool(name="const", bufs=1))
        xp = ctx.enter_context(tc.tile_pool(name="xp", bufs=1))
        sq = ctx.enter_context(tc.tile_pool(name="sq", bufs=2))
        sc = ctx.enter_context(tc.tile_pool(name="sc", bufs=1))
        xt = ctx.enter_context(tc.tile_pool(name="xt", bufs=1))
        wp = ctx.enter_context(tc.tile_pool(name="wp", bufs=1))
        ostp = ctx.enter_context(tc.tile_pool(name="ostp", bufs=4))
        tp = ctx.enter_context(tc.tile_pool(name="tp", bufs=2, space="PSUM"))
        gp = ctx.enter_context(tc.tile_pool(name="gp", bufs=1, space="PSUM"))
        mm = ctx.enter_context(tc.tile_pool(name="mm", bufs=5, space="PSUM"))

        # ---- input DMAs, issued up front in stream order ----
        # W8T first: the Gram (and so wscale) is the longest dependency
        # chain; x next for transposes; W16 groups last.
        w8t = wp.tile([P, NPAIR, 2, D], FP8)
        for s in range(4):
            j0 = s * (NPAIR // 4)
            nc.sync.dma_start(w8t[:, j0:j0 + NPAIR // 4],
                              w8t_d[:, j0:j0 + NPAIR // 4])
        x_sb = xp.tile([P, BT, D], BF16)
        nc.sync.dma_start(x_sb, x_r)
        w16 = wp.tile([P, DT, N], BF16)
        for g0, gw, _ in GROUPS:
            for t in range(DT):
                nc.sync.dma_start(w16[:, t, g0:g0 + gw],
                                  w16_r[:, t, g0:g0 + gw])

        # ---- preload both ACT tables before any data lands ----
        dum = sc.tile([P, 2], F32)
        dum2 = sc.tile([P, 2], F32)
        nc.scalar.activation(dum[:, 0:1], dum[:, 1:2], AF.Square)
        nc.scalar.activation(dum2[:, 0:1], dum[:, 0:1], AF.Sqrt)

        # ---- xscale = rsqrt(max(rowsumsq(x), eps)) on DVE (2-pass) ----
        xsqt = sq.tile([P, BT, D], BF16, tag="xsqt")
        nc.vector.tensor_tensor(xsqt, x_sb, x_sb, mybir.AluOpType.mult)
        xsq = sc.tile([P, BT], F32)
        nc.vector.reduce_sum(xsq, xsqt, axis=mybir.AxisListType.X)
        xmx = sc.tile([P, BT], F32)
        nc.vector.tensor_scalar_max(xmx, xsq, EPS)
        xsr = sc.tile([P, BT], F32)
        nc.scalar.sqrt(xsr, xmx)
        xsc = sc.tile([P, BT], F32)
        nc.vector.reciprocal(xsc, xsr)

        # ---- x^T via PE transpose (bf16, 1 cycle/row) ----
        identity = const.tile([P, P], BF16)
        make_identity(nc, identity)
        xtf = xt.tile([P, DT, BSH], BF16, tag="xtf")
        for dt in range(DT):
            for bt in range(BT):
                pt = tp.tile([P, P], BF16)
                nc.tensor.transpose(pt, x_sb[:, bt, dt * P:(dt + 1) * P],
                                    identity)
                nc.vector.tensor_copy(xtf[:, dt, bt * P:(bt + 1) * P], pt)

        # ---- W norms: PE Gram diagonal over the fp8 shadow ----
        # gps[db][i,j] accumulates sum_n W8T[n, db*128+i] * W8T[n, db*128+j];
        # its diagonal is sum_n W[d,n]^2 for d = db*128 + i.
        # PSUM start=True zeroing is bank-granular (2KB), so four 512B
        # accumulation regions in one bank cannot each use start=True:
        # every later start wipes the earlier regions' first pair.
        # Pre-zero the bank once and accumulate with start=False.
        gps = gp.tile([P, DT, P], F32)
        nc.vector.memset(gps, 0.0)
        for j in range(NPAIR):
            for db in range(DT):
                blk = w8t[:, j, :, db * P:(db + 1) * P]
                nc.tensor.matmul(gps[:, db, :], blk, blk,
                                 perf_mode=mybir.MatmulPerfMode.DoubleRow,
                                 start=False, stop=(j == NPAIR - 1),
                                 skip_group_check=True)
        wsq = sc.tile([P, DT], F32)
        for db in range(DT):
            dg = sq.tile([P, P], F32, tag="diag", name=f"dg{db}")
            nc.vector.tensor_tensor(dg, gps[:, db, :], identity,
                                    mybir.AluOpType.mult)
            nc.vector.reduce_sum(wsq[:, db:db + 1], dg,
                                 axis=mybir.AxisListType.X)
        nc.gpsimd.dma_start(sdbg_d, wsq)

        # ---- wscale = rsqrt(max(S, eps)) ----
        wmx = sc.tile([P, DT], F32)
        nc.vector.tensor_scalar_max(wmx, wsq, EPS)
        wsr = sc.tile([P, DT], F32)
        nc.scalar.sqrt(wsr, wmx)
        wsc = sc.tile([P, DT], F32)
        nc.vector.reciprocal(wsc, wsr)

        # ---- fold wscale into x^T ----
        xtr = xt.tile([P, DT, BSH], BF16, tag="xtr")
        for dt in range(DT):
            nc.vector.tensor_scalar_mul(xtr[:, dt, :], xtf[:, dt, :],
                                        wsc[:, dt:dt + 1])

        # ---- matmul: group outer (W arrival order), bt inner; dt outer
        # within a group so the stationary is reused across banks ----
        evict = [0]
        for g, (g0, gw, gix) in enumerate(GROUPS):
            grp = [CHUNKS[i] for i in gix]
            for bt in range(BT):
                pss = [mm.tile([P, CHUNK], F32, tag="ps", name=f"ps{c}")
                       for c in range(len(grp))]
                for dt in range(DT):
                    for c, (n0, nw) in enumerate(grp):
                        nc.tensor.matmul(
                            pss[c][:, :nw],
                            xtr[:, dt, bt * P:(bt + 1) * P],
                            w16[:, dt, n0:n0 + nw],
                            start=(dt == 0), stop=(dt == DT - 1))
                ost = ostp.tile([P, GRP * CHUNK], BF16, tag="ost")
                for c, (n0, nw) in enumerate(grp):
                    # GPSIMD cannot read PSUM: alternate ACT/DVE.
                    dst = ost[:, n0 - g0:n0 - g0 + nw]
                    if evict[0] % 2 == 0:
                        nc.scalar.activation(dst, pss[c][:, :nw], AF.Copy,
                                             scale=xsc[:, bt:bt + 1])
                    else:
                        nc.vector.tensor_scalar_mul(dst, pss[c][:, :nw],
                                                    xsc[:, bt:bt + 1])
                    evict[0] += 1
                # scalar = ACT HWDGE ring; gpsimd DMA is the slow SW queue
                nc.scalar.dma_start(o_r[:, bt, g0:g0 + gw], ost[:, :gw])

    nc.compile()
    return nc


LAST_RESULT = None


def kernel(x: np.ndarray, W: np.ndarray) -> np.ndarray:
    global LAST_RESULT
    x = np.ascontiguousarray(x, dtype=np.float32)
    W = np.ascontiguousarray(W, dtype=np.float32)
    assert x.shape == (B, D) and W.shape == (D, N)

    x16 = x.astype(ml_dtypes.bfloat16)
    W16 = np.ascontiguousarray(W.astype(ml_dtypes.bfloat16))

    # transposed fp8 shadow [N, D] -> pad to 6144 rows -> interleave to
    # [128, pair, 2, D]: partition p of pair j holds rows 256j+p and
    # 256j+128+p (DoubleRow k-tile pairing; self-products make the
    # pairing convention irrelevant).
    w8t = np.zeros((NTP, D), dtype=ml_dtypes.float8_e4m3)
    w8t[:N] = W.T.astype(ml_dtypes.float8_e4m3)
    w8t = np.ascontiguousarray(
        w8t.reshape(NPAIR, 2, P, D).transpose(2, 0, 1, 3))

    nc = _build()

    in_maps = [{"x": np.ascontiguousarray(x16[c * BSH:(c + 1) * BSH]),
                "W16": W16, "W8T": w8t}
               for c in range(NCORES)]

    res = run_bass_kernel_spmd(nc, in_maps, core_ids=list(range(NCORES)))
    LAST_RESULT = res
    out = np.concatenate([res.results[c]["out"] for c in range(NCORES)],
                         axis=0)
    return out.astype(np.float32)


# revision 24
# speedup vs baseline: 1.7059x; 1.0386x over previous
"""Trainium2 Bass kernel for nn_CosineSimilarityLayer.

out = l2norm_rows(x) @ l2norm_rows_over_N(W)       x:[4096,512]  W:[512,5994]

Math:  out[b,n] = xscale[b] * sum_d x[b,d] * wscale[d] * W[d,n]
  xscale[b] = rsqrt(max(sum_d x[b,d]^2, eps))   (folded into PSUM eviction)
  wscale[d] = rsqrt(max(sum_n W[d,n]^2, eps))   (folded into transposed x)

Sharding: data-parallel over batch - 8 cores x [512, 512] x-shards, W
replicated.  No collectives: a measured 2KB AllReduce costs ~40us here.

wscale gates every matmul, and the W-norm scan is too slow on the
vector engines (~14us of ACT/DVE/Pool time).  Instead the PE computes
it as a Gram diagonal: the host sends a transposed fp8e4m3 shadow of W
(pre-interleaved for DoubleRow), and the PE accumulates the 4 diagonal
128x128 blocks of W8T^T @ W8T over all 24 row-tile pairs - psum
diagonals are then sum_n W[d,n]^2.  A DVE identity-mask + reduce
extracts the diagonals.  Self-products make DoubleRow interleaving
correctness-free, and the PE (idle while waiting for wscale anyway)
chases the 3MB shadow DMA at line rate, so wscale is ready ~2us after
the shadow lands.  fp8 norm error is ~1e-3 relative on wscale ->
~1e-4 on out (gate 2e-2).

All IO bf16 (host casts, out upcast on host).  Matmul: group-outer /
bt-inner so W chunks are consumed ~3x slower than they land; PSUM
groups of 3 banks, 6 bufs; dt-outer inside a group reuses the
stationary.  Eviction alternates ACT/DVE; out DMA issued from GpSimd.
"""

import os
import sys
import types
from contextlib import ExitStack

import numpy as np


def _ensure_axon_hooks():
    """bass_utils' trace path imports antenv.axon_hooks, which some images
    lack.  Provide it (wired to the ctypes NTFF hook when available) so
    BASS_TRACE=1 profiles instead of crashing.  No-op when already present."""
    try:
        import antenv.axon_hooks  # noqa: F401
        return
    except ImportError:
        pass
    try:
        import antenv
    except ImportError:
        return
    m = types.ModuleType("antenv.axon_hooks")
    holder = {"h": None}
    m.set_axon_ntff_profile_hook = lambda h: holder.__setitem__("h", h)
    m.get_axon_ntff_profile_hook = lambda: holder["h"]
    sys.modules["antenv.axon_hooks"] = m
    antenv.axon_hooks = m
    try:
        from trn_agent_boot.trn_boot import _ntff_profile_via_ctypes
        so = "/opt/axon/libaxon_pjrt.so"
        if os.path.exists(so):
            m.set_axon_ntff_profile_hook(_ntff_profile_via_ctypes(so))
    except Exception:
        pass


_ensure_axon_hooks()

import ml_dtypes
import concourse.bass as bass
import concourse.tile as tile
from concourse import bacc, mybir
from concourse.bass_utils import run_bass_kernel_spmd
from concourse.masks import make_identity

F32 = mybir.dt.float32
BF16 = mybir.dt.bfloat16
FP8 = mybir.dt.float8e4
AF = mybir.ActivationFunctionType

B, D, N = 4096, 512, 5994
NCORES = 8
P = 128
BSH = B // NCORES          # 512 rows of x per core
BT = BSH // P              # 4 b-tiles
DT = D // P                # 4 d-tiles (contraction)
CHUNK = 512                # output n-chunk (one PSUM bank of fp32)
GRP = 3                    # chunks per PSUM group (6 mm banks + 2 tp banks)
EPS = 1e-12

NPAIR = 24                 # W8T row-tile pairs: 24*256 = 6144 >= 5994
NTP = NPAIR * 2 * P        # padded row count (6144)

CHUNKS = []
_n0 = 0
while _n0 < N:
    CHUNKS.append((_n0, min(CHUNK, N - _n0)))
    _n0 += CHUNK
NCH = len(CHUNKS)          # 12
# chunk-group partition: tiny leading groups so the first matmul only
# waits on 1 chunk of W16, then steady GRP-chunk groups
_GIDX = [[0], [1, 2], [3, 4, 5], [6, 7, 8], [9, 10], [11]]
GROUPS = []                # (start, width, [chunk indices])
for _ix in _GIDX:
    _c = [CHUNKS[i] for i in _ix]
    GROUPS.append((_c[0][0], _c[-1][0] + _c[-1][1] - _c[0][0], _ix))


def _build():
    nc = bacc.Bacc("TRN2", target_bir_lowering=False, debug=False,
                   num_devices=NCORES)

    x_d = nc.dram_tensor("x", [BSH, D], BF16, kind="ExternalInput").ap()
    w16_d = nc.dram_tensor("W16", [D, N], BF16, kind="ExternalInput").ap()
    # transposed fp8 shadow, host-interleaved to [128, pair, 2, D]
    w8t_d = nc.dram_tensor("W8T", [P, NPAIR, 2, D], FP8,
                           kind="ExternalInput").ap()
    o_d = nc.dram_tensor("out", [BSH, N], BF16, kind="ExternalOutput").ap()
    sdbg_d = nc.dram_tensor("SDBG", [P, DT], F32, kind="ExternalOutput").ap()

    x_r = x_d.rearrange("(t p) d -> p t d", p=P)        # [128, 4, 512]
    w16_r = w16_d.rearrange("(t p) n -> p t n", p=P)    # [128, 4, 5994]
    o_r = o_d.rearrange("(t p) n -> p t n", p=P)        # [128, 4, 5994]

    with tile.TileContext(nc) as tc, ExitStack() as ctx:
        const = ctx.enter_context(tc.tile_pool(name="const", bufs=1))
        xp = ctx.enter_context(tc.tile_pool(name="xp", bufs=1))
        sq = ctx.enter_context(tc.tile_pool(name="sq", bufs=2))
        sc = ctx.enter_context(tc.tile_pool(name="sc", bufs=1))
        xt = ctx.enter_context(tc.tile_pool(name="xt", bufs=1))
        wp = ctx.enter_context(tc.tile_pool(name="wp", bufs=1))
        ostp = ctx.enter_context(tc.tile_pool(name="ostp", bufs=4))
        tp = ctx.enter_context(tc.tile_pool(name="tp", bufs=2, space="PSUM"))
        gp = ctx.enter_context(tc.tile_pool(name="gp", bufs=1, space="PSUM"))
        mm = ctx.enter_context(tc.tile_pool(name="mm", bufs=5, space="PSUM"))

        # ---- input DMAs, issued up front in stream order ----
        # W8T first: the Gram (and so wscale) is the longest dependency
        # chain; x next for transposes; W16 groups last.
        w8t = wp.tile([P, NPAIR, 2, D], FP8)
        for s in range(4):
            j0 = s * (NPAIR // 4)
            nc.sync.dma_start(w8t[:, j0:j0 + NPAIR // 4],
                              w8t_d[:, j0:j0 + NPAIR // 4])
        x_sb = xp.tile([P, BT, D], BF16)
        nc.sync.dma_start(x_sb, x_r)
        w16 = wp.tile([P, DT, N], BF16)
        for g0, gw, _ in GROUPS:
            for t in range(DT):
                nc.sync.dma_start(w16[:, t, g0:g0 + gw],
                                  w16_r[:, t, g0:g0 + gw])

        # ---- preload both ACT tables before any data lands ----
        dum = sc.tile([P, 2], F32)
        dum2 = sc.tile([P, 2], F32)
        nc.scalar.activation(dum[:, 0:1], dum[:, 1:2], AF.Square)
        nc.scalar.activation(dum2[:, 0:1], dum[:, 0:1], AF.Sqrt)
        identity = const.tile([P, P], BF16)
        make_identity(nc, identity)

        # ---- W norms: PE Gram diagonal over the fp8 shadow ----
        # Emitted FIRST on the PE (engines execute in program order): the
        # gram paces the wscale critical chain and must not sit behind
        # the x transposes in the PE stream.
        # gps[db][i,j] accumulates sum_n W8T[n, db*128+i] * W8T[n, db*128+j];
        # its diagonal is sum_n W[d,n]^2 for d = db*128 + i.
        # PSUM start=True zeroing is bank-granular (2KB), so four 512B
        # accumulation regions in one bank cannot each use start=True:
        # every later start wipes the earlier regions' first pair.
        # Pre-zero the bank once and accumulate with start=False.
        gps = gp.tile([P, DT, P], F32)
        nc.vector.memset(gps, 0.0)
        for j in range(NPAIR):
            for db in range(DT):
                blk = w8t[:, j, :, db * P:(db + 1) * P]
                nc.tensor.matmul(gps[:, db, :], blk, blk,
                                 perf_mode=mybir.MatmulPerfMode.DoubleRow,
                                 start=False, stop=(j == NPAIR - 1),
                                 skip_group_check=True)

        # ---- xscale = rsqrt(max(rowsumsq(x), eps)) on ACT (fused) ----
        xsq = sc.tile([P, BT], F32)
        for bt in range(BT):
            trx = sq.tile([P, D], BF16, tag="trx")
            nc.scalar.activation(trx, x_sb[:, bt, :], AF.Square,
                                 accum_out=xsq[:, bt:bt + 1])

        # ---- x^T via PE transpose (bf16, 1 cycle/row) ----
        xtf = xt.tile([P, DT, BSH], BF16, tag="xtf")
        for dt in range(DT):
            for bt in range(BT):
                pt = tp.tile([P, P], BF16)
                nc.tensor.transpose(pt, x_sb[:, bt, dt * P:(dt + 1) * P],
                                    identity)
                nc.vector.tensor_copy(xtf[:, dt, bt * P:(bt + 1) * P], pt)

        # ---- gram diag extract + both rsqrt chains ----
        wsq = sc.tile([P, DT], F32)
        for db in range(DT):
            dg = sq.tile([P, P], F32, tag="diag", name=f"dg{db}")
            nc.vector.tensor_tensor(dg, gps[:, db, :], identity,
                                    mybir.AluOpType.mult)
            nc.vector.reduce_sum(wsq[:, db:db + 1], dg,
                                 axis=mybir.AxisListType.X)
        nc.gpsimd.dma_start(sdbg_d, wsq)

        wmx = sc.tile([P, DT], F32)
        nc.vector.tensor_scalar_max(wmx, wsq, EPS)
        wsr = sc.tile([P, DT], F32)
        nc.scalar.sqrt(wsr, wmx)
        wsc = sc.tile([P, DT], F32)
        nc.vector.reciprocal(wsc, wsr)

        xmx = sc.tile([P, BT], F32)
        nc.vector.tensor_scalar_max(xmx, xsq, EPS)
        xsr = sc.tile([P, BT], F32)
        nc.scalar.sqrt(xsr, xmx)
        xsc = sc.tile([P, BT], F32)
        nc.vector.reciprocal(xsc, xsr)

        # ---- fold wscale into x^T ----
        xtr = xt.tile([P, DT, BSH], BF16, tag="xtr")
        for dt in range(DT):
            nc.vector.tensor_scalar_mul(xtr[:, dt, :], xtf[:, dt, :],
                                        wsc[:, dt:dt + 1])

        # ---- matmul: group outer (W arrival order), bt inner; dt outer
        # within a group so the stationary is reused across banks ----
        evict = [0]
        for g, (g0, gw, gix) in enumerate(GROUPS):
            grp = [CHUNKS[i] for i in gix]
            for bt in range(BT):
                pss = [mm.tile([P, CHUNK], F32, tag="ps", name=f"ps{c}")
                       for c in range(len(grp))]
                for dt in range(DT):
                    for c, (n0, nw) in enumerate(grp):
                        nc.tensor.matmul(
                            pss[c][:, :nw],
                            xtr[:, dt, bt * P:(bt + 1) * P],
                            w16[:, dt, n0:n0 + nw],
                            start=(dt == 0), stop=(dt == DT - 1))
                ost = ostp.tile([P, GRP * CHUNK], BF16, tag="ost")
                for c, (n0, nw) in enumerate(grp):
                    # GPSIMD cannot read PSUM: alternate ACT/DVE.
                    dst = ost[:, n0 - g0:n0 - g0 + nw]
                    if evict[0] % 2 == 0:
                        nc.scalar.activation(dst, pss[c][:, :nw], AF.Copy,
                                             scale=xsc[:, bt:bt + 1])
                    else:
                        nc.vector.tensor_scalar_mul(dst, pss[c][:, :nw],
                                                    xsc[:, bt:bt + 1])
                    evict[0] += 1
                # scalar = ACT HWDGE ring; gpsimd DMA is the slow SW queue
                nc.scalar.dma_start(o_r[:, bt, g0:g0 + gw], ost[:, :gw])

    nc.compile()
    return nc


LAST_RESULT = None


def kernel(x: np.ndarray, W: np.ndarray) -> np.ndarray:
    global LAST_RESULT
    x = np.ascontiguousarray(x, dtype=np.float32)
    W = np.ascontiguousarray(W, dtype=np.float32)
    assert x.shape == (B, D) and W.shape == (D, N)

    x16 = x.astype(ml_dtypes.bfloat16)
    W16 = np.ascontiguousarray(W.astype(ml_dtypes.bfloat16))

    # transposed fp8 shadow [N, D] -> pad to 6144 rows -> interleave to
    # [128, pair, 2, D]: partition p of pair j holds rows 256j+p and
    # 256j+128+p (DoubleRow k-tile pairing; self-products make the
    # pairing convention irrelevant).
    w8t = np.zeros((NTP, D), dtype=ml_dtypes.float8_e4m3)
    w8t[:N] = W.T.astype(ml_dtypes.float8_e4m3)
    w8t = np.ascontiguousarray(
        w8t.reshape(NPAIR, 2, P, D).transpose(2, 0, 1, 3))

    nc = _build()

    in_maps = [{"x": np.ascontiguousarray(x16[c * BSH:(c + 1) * BSH]),
                "W16": W16, "W8T": w8t}
               for c in range(NCORES)]

    res = run_bass_kernel_spmd(nc, in_maps, core_ids=list(range(NCORES)))
    LAST_RESULT = res
    out = np.concatenate([res.results[c]["out"] for c in range(NCORES)],
                         axis=0)
    return out.astype(np.float32)
